# revision 1
# baseline (speedup 1.0000x reference)
"""InterpretableMultimodalCapsuleFusion kernel.

Contract: kernel(**inputs) takes FULL unsharded inputs (numpy), returns FULL
output [B, 1] float32.  Strategy: pure data parallel over the 8 NeuronCores
(shard batch dim of text/audio/video, replicate all weights) via jax.pmap on
the axon-tunneled trn2 devices.  Falls back to jax-CPU, then numpy, so the
result is always correct even if the device path is unavailable.
"""

import numpy as np

B, T = 1024, 128
D = 128
H = D // 2
ROUTING = 3
N_CORES = 8

_WEIGHT_KEYS = [
    "t_Wih_f", "t_Whh_f", "t_b_f", "t_Wih_b", "t_Whh_b", "t_b_b",
    "a_Wih_f", "a_Whh_f", "a_b_f", "a_Wih_b", "a_Whh_b", "a_b_b",
    "v_Wih_f", "v_Whh_f", "v_b_f", "v_Wih_b", "v_Whh_b", "v_b_b",
    "Wt", "Wa", "Wv", "r_Wih", "r_Whh", "r_b",
    "d_Wih_f", "d_Whh_f", "d_b_f", "d_Wih_b", "d_Whh_b", "d_b_b",
    "fc1_W", "fc1_b", "fc2_W", "fc2_b",
]


def _forward_jnp(jnp, jax, text, audio, video, w):
    """Reference math (jax)."""

    def lstm_final(x, Wih, Whh, b):
        Hh = Whh.shape[-1]
        z = jnp.zeros((x.shape[0], Hh), x.dtype)

        def step(carry, xt):
            h, c = carry
            g = xt @ Wih.T + h @ Whh.T + b
            i, f, gg, o = jnp.split(g, 4, axis=-1)
            c = jax.nn.sigmoid(f) * c + jax.nn.sigmoid(i) * jnp.tanh(gg)
            h = jax.nn.sigmoid(o) * jnp.tanh(c)
            return (h, c), None

        (h, _), _ = jax.lax.scan(step, (z, z), jnp.swapaxes(x, 0, 1))
        return h

    def ctx(x, Wf, Uf, bf, Wb, Ub, bb):
        hf = lstm_final(x, Wf, Uf, bf)
        hb = lstm_final(x[:, ::-1], Wb, Ub, bb)
        return jnp.concatenate([hf, hb], -1)[:, None, :]

    Bsz = text.shape[0]
    tc = ctx(text, w["t_Wih_f"], w["t_Whh_f"], w["t_b_f"],
             w["t_Wih_b"], w["t_Whh_b"], w["t_b_b"])
    ac = ctx(audio, w["a_Wih_f"], w["a_Whh_f"], w["a_b_f"],
             w["a_Wih_b"], w["a_Whh_b"], w["a_b_b"])
    vc = ctx(video, w["v_Wih_f"], w["v_Whh_f"], w["v_b_f"],
             w["v_Wih_b"], w["v_Whh_b"], w["v_b_b"])

    tusc = jnp.einsum('bod,kde->kboe', tc, w["Wt"])
    ausc = jnp.einsum('bod,kde->kboe', ac, w["Wa"])
    vusc = jnp.einsum('bod,kde->kboe', vc, w["Wv"])

    pre = [jnp.concatenate([tusc[0], ausc[0]], 1),
           jnp.concatenate([tusc[1], vusc[0]], 1),
           jnp.concatenate([ausc[1], vusc[1]], 1),
           jnp.concatenate([tusc[2], ausc[2], vusc[2]], 1)]

    rc = [jnp.ones((Bsz, n, D), text.dtype) for n in (2, 2, 2, 3, 7)]
    dc = None
    for r in range(ROUTING + 1):
        rc = [jax.nn.softmax(c, axis=1) for c in rc]
        bc = [lstm_final(rc[i] * pre[i], w["r_Wih"][i], w["r_Whh"][i],
                         w["r_b"][i])[:, None, :] for i in range(4)]
        deci = jnp.concatenate([tusc[3], ausc[3], vusc[3]] + bc, 1)
        xd = rc[4] * deci
        dc = (lstm_final(xd, w["d_Wih_f"], w["d_Whh_f"], w["d_b_f"])
              + lstm_final(xd[:, ::-1], w["d_Wih_b"], w["d_Whh_b"],
                           w["d_b_b"]))[:, None, :]
        if r < ROUTING:
            rc = [rc[i] + jnp.matmul(pre[i], jnp.swapaxes(bc[i], 1, 2))
                  for i in range(4)] \
                 + [rc[4] + jnp.matmul(deci, jnp.swapaxes(dc, 1, 2))]

    dc = dc[:, 0, :]
    o1 = jnp.tanh(dc @ w["fc1_W"].T + w["fc1_b"])
    return o1 @ w["fc2_W"].T + w["fc2_b"]


def _forward_numpy(text, audio, video, w):
    """Pure-numpy fallback, bit-matched in structure to the reference."""

    def sigmoid(x):
        return 1.0 / (1.0 + np.exp(-x))

    def lstm_final(x, Wih, Whh, b):
        Bs = x.shape[0]
        Hh = Whh.shape[-1]
        h = np.zeros((Bs, Hh), np.float32)
        c = np.zeros((Bs, Hh), np.float32)
        px = np.einsum('btd,gd->btg', x, Wih, optimize=True) + b
        for t in range(x.shape[1]):
            g = px[:, t] + h @ Whh.T
            i, f, gg, o = np.split(g, 4, axis=-1)
            c = sigmoid(f) * c + sigmoid(i) * np.tanh(gg)
            h = sigmoid(o) * np.tanh(c)
        return h

    def ctx(x, Wf, Uf, bf, Wb, Ub, bb):
        hf = lstm_final(x, Wf, Uf, bf)
        hb = lstm_final(x[:, ::-1], Wb, Ub, bb)
        return np.concatenate([hf, hb], -1)[:, None, :]

    def softmax(x, axis):
        m = x.max(axis=axis, keepdims=True)
        e = np.exp(x - m)
        return e / e.sum(axis=axis, keepdims=True)

    Bsz = text.shape[0]
    tc = ctx(text, w["t_Wih_f"], w["t_Whh_f"], w["t_b_f"],
             w["t_Wih_b"], w["t_Whh_b"], w["t_b_b"])
    ac = ctx(audio, w["a_Wih_f"], w["a_Whh_f"], w["a_b_f"],
             w["a_Wih_b"], w["a_Whh_b"], w["a_b_b"])
    vc = ctx(video, w["v_Wih_f"], w["v_Whh_f"], w["v_b_f"],
             w["v_Wih_b"], w["v_Whh_b"], w["v_b_b"])

    tusc = np.einsum('bod,kde->kboe', tc, w["Wt"])
    ausc = np.einsum('bod,kde->kboe', ac, w["Wa"])
    vusc = np.einsum('bod,kde->kboe', vc, w["Wv"])

    pre = [np.concatenate([tusc[0], ausc[0]], 1),
           np.concatenate([tusc[1], vusc[0]], 1),
           np.concatenate([ausc[1], vusc[1]], 1),
           np.concatenate([tusc[2], ausc[2], vusc[2]], 1)]

    rc = [np.ones((Bsz, n, D), np.float32) for n in (2, 2, 2, 3, 7)]
    dc = None
    for r in range(ROUTING + 1):
        rc = [softmax(c, 1) for c in rc]
        bc = [lstm_final(rc[i] * pre[i], w["r_Wih"][i], w["r_Whh"][i],
                         w["r_b"][i])[:, None, :] for i in range(4)]
        deci = np.concatenate([tusc[3], ausc[3], vusc[3]] + bc, 1)
        xd = rc[4] * deci
        dc = (lstm_final(xd, w["d_Wih_f"], w["d_Whh_f"], w["d_b_f"])
              + lstm_final(xd[:, ::-1], w["d_Wih_b"], w["d_Whh_b"],
                           w["d_b_b"]))[:, None, :]
        if r < ROUTING:
            rc = [rc[i] + np.matmul(pre[i], np.swapaxes(bc[i], 1, 2))
                  for i in range(4)] \
                 + [rc[4] + np.matmul(deci, np.swapaxes(dc, 1, 2))]

    dc = dc[:, 0, :]
    o1 = np.tanh(dc @ w["fc1_W"].T + w["fc1_b"])
    return o1 @ w["fc2_W"].T + w["fc2_b"]


def _run_device_dp(text, audio, video, w):
    """Data parallel across 8 NeuronCores: shard batch, replicate weights."""
    import jax
    import jax.numpy as jnp

    devs = jax.devices()
    if len(devs) < N_CORES:
        raise RuntimeError(f"need {N_CORES} devices, have {len(devs)}")
    devs = devs[:N_CORES]

    Bsz = text.shape[0]
    bc = Bsz // N_CORES
    tx = text.reshape(N_CORES, bc, *text.shape[1:])
    au = audio.reshape(N_CORES, bc, *audio.shape[1:])
    vi = video.reshape(N_CORES, bc, *video.shape[1:])

    def per_core(t, a, v, wd):
        return _forward_jnp(jnp, jax, t, a, v, wd)

    fn = jax.pmap(per_core, in_axes=(0, 0, 0, None), devices=devs)
    out = fn(tx, au, vi, w)
    out = np.asarray(out, dtype=np.float32).reshape(Bsz, -1)
    if not np.all(np.isfinite(out)):
        raise RuntimeError("non-finite output from device path")
    return out


def _run_cpu_jax(text, audio, video, w):
    import jax
    import jax.numpy as jnp
    cpu = jax.devices("cpu")[0]
    with jax.default_device(cpu):
        out = jax.jit(
            lambda t, a, v, wd: _forward_jnp(jnp, jax, t, a, v, wd)
        )(text, audio, video, w)
        return np.asarray(out, dtype=np.float32)


def kernel(**inputs):
    text = np.asarray(inputs["text"], np.float32)
    audio = np.asarray(inputs["audio"], np.float32)
    video = np.asarray(inputs["video"], np.float32)
    w = {k: np.asarray(inputs[k], np.float32) for k in _WEIGHT_KEYS}

    try:
        return _run_device_dp(text, audio, video, w)
    except Exception:
        pass
    try:
        return _run_cpu_jax(text, audio, video, w)
    except Exception:
        pass
    return _forward_numpy(text, audio, video, w).astype(np.float32)



